# revision 1
# baseline (speedup 1.0000x reference)
"""Trainium2 Bass kernel: 5x5 reflect-padded box-filter mean (LocalMean).

Full input:  image (32, 3, 512, 512) f32
Full output: same shape; out[r,c] = mean of the 5x5 window of the
reflect-padded image.

Strategy (pure data parallel over 8 NeuronCores, 4 images per core):
- Host pre-pads H and W by 2 with reflect -> (4, 3, 516, 516) per core.
- On-chip the filter is separable:
  * vertical 5-tap sum via TensorE banded matmuls (constant lower-band
    weight tile, 1/25-scaled; row blocks of 124 output rows so each
    block's 128 input rows live in a single SBUF tile -> one matmul),
  * horizontal 5-tap sum via one DVE reduce (window head) plus one DVE
    tensor_tensor_scan per block: H[j] = (V[j+4] + H[j-1]) - V[j-1].
- ScalarE copies the PSUM intermediate to SBUF (scan operands may not
  both live in PSUM); DMA (HBM ~13MB in + 12.6MB out per core) is the
  roofline bottleneck. GPSIMD is intentionally unused (2-input
  elementwise there is several times slower than DVE and contends for
  the DVE SBUF port).
"""

import numpy as np

N_CORES = 8
B, C, H, W = 32, 3, 512, 512
PB = B // N_CORES          # images per core
PAD = 2
HP, WP = H + 2 * PAD, W + 2 * PAD   # 516

# Output-row blocks of 124 (last 16): input rows [124b, 124b+128) per
# block all sit in one 128-partition tile, so the vertical matmul needs
# no cross-tile tail accumulation.
BLOCKS = [(0, 124), (124, 124), (248, 124), (372, 124), (496, 16)]

_CACHE = {}
# Experiment switches (default = the shipped configuration).
_CFG = {}


def _band_weights():
    # W[k, m] = 1/25 for 0 <= k-m <= 4: vertical 5-tap window starting at
    # output row m reads input rows m..m+4 of the padded block.
    def band(K, M):
        k = np.arange(K)[:, None]
        m = np.arange(M)[None, :]
        return (((k - m) >= 0) & ((k - m) <= 4)).astype(np.float32) / 25.0
    return band(128, 124), band(20, 16)


def _build(reps=1):
    import concourse.bacc as bacc
    import concourse.tile as tile
    from concourse import mybir

    f32 = mybir.dt.float32
    nc = bacc.Bacc("TRN2", target_bir_lowering=False, debug=False,
                   num_devices=N_CORES)
    x = nc.dram_tensor("x", [PB, C, HP, WP], f32, kind="ExternalInput").ap()
    wd = nc.dram_tensor("wd", [128, 124], f32, kind="ExternalInput").ap()
    wl = nc.dram_tensor("wl", [20, 16], f32, kind="ExternalInput").ap()
    y = nc.dram_tensor("y", [PB, C, H, W], f32, kind="ExternalOutput").ap()

    LOOKAHEAD = 3  # channel-images of input prefetched ahead of compute

    with tile.TileContext(nc) as tc:
        with (
            tc.tile_pool(name="wp", bufs=1) as wp,
            tc.tile_pool(name="xp", bufs=4 * (LOOKAHEAD + 2)) as xp,
            tc.tile_pool(name="xtp", bufs=LOOKAHEAD + 2) as xtp,
            tc.tile_pool(name="vp", bufs=4, space="PSUM") as vp,
            tc.tile_pool(name="vsp", bufs=6) as vsp,
            tc.tile_pool(name="op", bufs=8) as op,
        ):
            d_t = wp.tile([128, 124], f32)
            nc.sync.dma_start(d_t[:], wd[:, :])
            l_t = wp.tile([20, 16], f32)
            nc.sync.dma_start(l_t[:], wl[:, :])

            cis = [(n, c) for _ in range(reps)
                   for n in range(PB) for c in range(C)]
            loaded = {}  # step index -> list of 5 X tiles

            def load(s):
                n, c = cis[s]
                xts = []
                for b, (r0, h) in enumerate(BLOCKS):
                    kh = 128 if h == 124 else 20
                    pool = xp if kh == 128 else xtp
                    t = pool.tile([kh, WP], f32)
                    nc.sync.dma_start(t[:], x[n, c, r0:r0 + kh, :])
                    xts.append(t)
                loaded[s] = xts

            for s in range(min(LOOKAHEAD, len(cis))):
                load(s)

            for s, (n, c) in enumerate(cis):
                if s + LOOKAHEAD < len(cis):
                    load(s + LOOKAHEAD)
                xts = loaded.pop(s)

                for b, (r0, h) in enumerate(BLOCKS):
                    w_t = d_t if h == 124 else l_t
                    xt = xts[b]
                    v = vp.tile([128, WP], f32)
                    # V[m, :] = sum_{d=0..4} X[m+d, :] / 25, via banded
                    # matmul; N split at the PSUM bank boundary (fp32
                    # matmul N <= 512).
                    nc.tensor.matmul(v[0:h, 0:512], w_t[:], xt[:, 0:512],
                                     start=True, stop=True)
                    nc.tensor.matmul(v[0:h, 512:516], w_t[:], xt[:, 512:516],
                                     start=True, stop=True)

                    # PSUM -> SBUF once on the otherwise-idle ScalarE: DVE
                    # SBUF reads are 62 cycles/op cheaper than PSUM reads,
                    # so routing both scan operands through SBUF wins over
                    # reading V from PSUM directly (measured in sim).
                    vs = vsp.tile([128, WP], f32)
                    nc.scalar.copy(vs[0:h, :], v[0:h, :])

                    # Horizontal 5-tap sliding window on DVE:
                    #   H[0] = sum(Vs[0:5]);  H[j] = H[j-1] + Vs[j+4] - Vs[j-1]
                    o = op.tile([128, W], f32)
                    nc.vector.reduce_sum(o[0:h, 0:1], vs[0:h, 0:5],
                                         axis=mybir.AxisListType.X)
                    nc.vector.tensor_tensor_scan(
                        o[0:h, 1:512], vs[0:h, 5:516], vs[0:h, 0:511],
                        o[0:h, 0:1],
                        mybir.AluOpType.add, mybir.AluOpType.subtract)
                    # Output DMAs alternate between the two HWDGE queues:
                    # DMA *issue* costs ~0.65us per dma_start on an in-order
                    # sequencer, so issue work must be spread — SP carries
                    # the input DMAs, ACT the PSUM->SBUF copies, and each
                    # takes half the output issues to balance at ~58us.
                    dma_eng = nc.scalar if (s * 5 + b) % 2 == 0 else nc.sync
                    dma_eng.dma_start(y[n, c, r0:r0 + h, :], o[0:h, :])

    nc.compile()
    return nc


def _get_nc(reps=1):
    key = ("nc", reps)
    if key not in _CACHE:
        _CACHE[key] = _build(reps)
    return _CACHE[key]


def _shard_inputs(image: np.ndarray):
    image = np.ascontiguousarray(np.asarray(image, dtype=np.float32))
    padded = np.pad(image, ((0, 0), (0, 0), (PAD, PAD), (PAD, PAD)),
                    mode="reflect")
    d, dl = _band_weights()
    in_maps = []
    for i in range(N_CORES):
        in_maps.append({
            "x": np.ascontiguousarray(padded[i * PB:(i + 1) * PB]),
            "wd": d,
            "wl": dl,
        })
    return in_maps


def kernel(image: np.ndarray) -> np.ndarray:
    from concourse import bass_utils

    nc = _get_nc()
    in_maps = _shard_inputs(image)
    res = bass_utils.run_bass_kernel_spmd(nc, in_maps,
                                          core_ids=list(range(N_CORES)))
    return np.concatenate([res.results[i]["y"] for i in range(N_CORES)], axis=0)



# revision 3
# speedup vs baseline: 3.1255x; 3.1255x over previous
"""Trainium2 Bass kernel: 5x5 reflect-padded box-filter mean (LocalMean).

Full input:  image (32, 3, 512, 512) f32
Full output: same shape; out[r,c] = mean of the 5x5 window of the
reflect-padded image.

Strategy (pure data parallel over 8 NeuronCores, 4 images per core):
- Host pre-pads H and W by 2 with reflect, moves channels inside rows
  ([PB, HP, C*WP]) and casts to bf16 -> HBM input traffic halves and a
  single DMA per (image, row-block) carries all 3 channels.
- On-chip the filter is separable:
  * vertical 5-tap sum via TensorE banded matmuls in bf16 (1 cycle/row
    vs 4 for fp32), weight band pre-scaled by 1/25; row blocks of 124
    output rows so the 128 input rows fit one SBUF tile,
  * ScalarE copies PSUM f32 -> SBUF bf16 (the only PSUM drain),
  * horizontal 5-tap sum via ONE DVE tensor_tensor_scan per block that
    runs across all 3 channels back-to-back: the recurrence
    H[t] = H[t-1] + V[t] - V[t-5] telescopes exactly (fp32 internal
    state), so window sums spanning a channel boundary are garbage but
    self-cancel 5 steps later and are simply never stored.
- Output stays bf16 ([PB, H, C*W]) and is upcast on host -> output HBM
  traffic halves too. Total HBM/core ~12.7 MB vs ~25.4 MB for f32.
"""

import numpy as np

N_CORES = 8
B, C, H, W = 32, 3, 512, 512
PB = B // N_CORES          # images per core
PAD = 2
HP, WP = H + 2 * PAD, W + 2 * PAD   # 516
FW = C * WP                # 1548: in-tile free width (3 channels)
FO = C * W                 # 1536: out free width

# Output-row blocks of 124 (last 16): input rows [r0, r0+h+4) per block
# sit in one 128-partition tile, so the vertical matmul needs no
# cross-tile tail accumulation.
BLOCKS = [(0, 124), (124, 124), (248, 124), (372, 124), (496, 16)]

_CACHE = {}


def _band_weights():
    # W[k, m] = 1/25 for 0 <= k-m <= 4: vertical 5-tap window starting at
    # output row m reads input rows m..m+4 of the padded block.
    def band(K, M):
        k = np.arange(K)[:, None]
        m = np.arange(M)[None, :]
        return (((k - m) >= 0) & ((k - m) <= 4)).astype(np.float32) / 25.0
    return band(128, 124), band(20, 16)


def _build(reps=1):
    import concourse.bacc as bacc
    import concourse.tile as tile
    from concourse import mybir

    f32 = mybir.dt.float32
    bf16 = mybir.dt.bfloat16
    nc = bacc.Bacc("TRN2", target_bir_lowering=False, debug=False,
                   num_devices=N_CORES)
    x = nc.dram_tensor("x", [PB, HP, FW], bf16, kind="ExternalInput").ap()
    wd = nc.dram_tensor("wd", [128, 124], bf16, kind="ExternalInput").ap()
    wl = nc.dram_tensor("wl", [20, 16], bf16, kind="ExternalInput").ap()
    y = nc.dram_tensor("y", [PB, H, FO], bf16, kind="ExternalOutput").ap()

    LOOKAHEAD = 4  # row-blocks of input prefetched ahead of compute

    with tile.TileContext(nc) as tc:
        with (
            tc.tile_pool(name="wp", bufs=1) as wp,
            tc.tile_pool(name="xp", bufs=LOOKAHEAD + 3) as xp,
            tc.tile_pool(name="vp", bufs=2, space="PSUM") as vp,
            tc.tile_pool(name="vsp", bufs=3) as vsp,
            tc.tile_pool(name="op", bufs=6) as op,
        ):
            d_t = wp.tile([128, 124], bf16)
            nc.sync.dma_start(d_t[:], wd[:, :])
            l_t = wp.tile([20, 16], bf16)
            nc.sync.dma_start(l_t[:], wl[:, :])

            steps = [(n, b) for _ in range(reps)
                     for n in range(PB) for b in range(len(BLOCKS))]
            loaded = {}  # step index -> X tile

            def load(s):
                n, b = steps[s]
                r0, h = BLOCKS[b]
                kh = h + 4
                t = xp.tile([128, FW], bf16)
                nc.sync.dma_start(t[0:kh, :], x[n, r0:r0 + kh, :])
                loaded[s] = t

            for s in range(min(LOOKAHEAD, len(steps))):
                load(s)

            for s, (n, b) in enumerate(steps):
                if s + LOOKAHEAD < len(steps):
                    load(s + LOOKAHEAD)
                xt = loaded.pop(s)
                r0, h = BLOCKS[b]
                kh = h + 4
                w_t = d_t if h == 124 else l_t

                # V[m, t] = sum_{d=0..4} X[m+d, t] / 25 via banded matmul;
                # N split at PSUM bank boundaries (512 f32 per bank).
                v = vp.tile([128, FW], f32)
                for c0 in range(0, FW, 512):
                    c1 = min(c0 + 512, FW)
                    nc.tensor.matmul(v[0:h, c0:c1], w_t[0:kh, 0:h],
                                     xt[0:kh, c0:c1], start=True, stop=True)

                # Single PSUM drain, f32 -> bf16 (scan operands must not
                # both be in PSUM, and DVE reads SBUF cheaper anyway).
                vs = vsp.tile([128, FW], bf16)
                nc.scalar.copy(vs[0:h, :], v[0:h, :])

                # Horizontal 5-tap sliding window, one scan across all 3
                # channels: o[t] = sum(vs[t-4..t]); channel c's valid
                # outputs are cols 516c+4 .. 516c+515.
                o = op.tile([128, FW], bf16)
                with nc.allow_low_precision(
                        reason="5-tap window sums of ~0.5-magnitude values; "
                               "scan state is fp32 internally, tol is 2e-2"):
                    nc.vector.reduce_sum(o[0:h, 4:5], vs[0:h, 0:5],
                                         axis=mybir.AxisListType.X)
                    nc.vector.tensor_tensor_scan(
                        o[0:h, 5:FW], vs[0:h, 5:FW], vs[0:h, 0:FW - 5],
                        o[0:h, 4:5],
                        mybir.AluOpType.add, mybir.AluOpType.subtract)

                o3 = o.rearrange("p (c w) -> p c w", c=C)
                nc.sync.dma_start(y[n, r0:r0 + h, :], o3[0:h, :, 4:WP])

    nc.compile()
    return nc


def _get_nc(reps=1):
    key = ("nc", reps)
    if key not in _CACHE:
        _CACHE[key] = _build(reps)
    return _CACHE[key]


def _shard_inputs(image: np.ndarray):
    import ml_dtypes

    image = np.ascontiguousarray(np.asarray(image, dtype=np.float32))
    padded = np.pad(image, ((0, 0), (0, 0), (PAD, PAD), (PAD, PAD)),
                    mode="reflect")
    # [B, C, HP, WP] -> [B, HP, C, WP] -> bf16 [B, HP, C*WP]
    xh = np.ascontiguousarray(padded.transpose(0, 2, 1, 3)) \
        .astype(ml_dtypes.bfloat16).reshape(B, HP, FW)
    d, dl = _band_weights()
    d = d.astype(ml_dtypes.bfloat16)
    dl = dl.astype(ml_dtypes.bfloat16)
    in_maps = []
    for i in range(N_CORES):
        in_maps.append({
            "x": np.ascontiguousarray(xh[i * PB:(i + 1) * PB]),
            "wd": d,
            "wl": dl,
        })
    return in_maps


def kernel(image: np.ndarray) -> np.ndarray:
    from concourse import bass_utils

    nc = _get_nc()
    in_maps = _shard_inputs(image)
    res = bass_utils.run_bass_kernel_spmd(nc, in_maps,
                                          core_ids=list(range(N_CORES)))
    ys = np.concatenate([np.asarray(res.results[i]["y"])
                         for i in range(N_CORES)], axis=0)
    # [B, H, C*W] bf16 -> f32 [B, C, H, W]
    out = ys.astype(np.float32).reshape(B, H, C, W).transpose(0, 2, 1, 3)
    return np.ascontiguousarray(out)
